# revision 19
# baseline (speedup 1.0000x reference)
"""Trainium2 Bass kernel for CropConLoss (supervised-contrastive style loss).

Contract: kernel(**inputs) takes the FULL unsharded inputs
(protos [64,128] f32, proj2/proj3 [4096,128] f32, target2/target3 [4096] i64)
and returns the FULL output (scalar f32 mean loss), running the compute on
8 NeuronCores via bass_utils.run_bass_kernel_spmd.

Strategy (data-parallel over the M=8192 rows of feats, ACT-roofline design):
  - Host sorts the 8192 rows by class label and l2-normalizes them (and the
    protos) in f32, so the device needs no sqrt/reciprocal and a single
    constant exp scale of 1/tau.
  - Each core owns 1024 query rows. Layout is [query-partition, key-free]:
    per q-tile (128 queries) the stationary operand is the query block of
    keysT and the 8192 keys stream through the PE in 512-col chunks.
  - exp runs on ACT over [128, 2048] PSUM chunks into f32 SBUF tiles; ACT
    is the roofline engine (~64us of exp). Row-sums are per-chunk DVE
    reduces (NOT accum_out - its hidden ACTIVATION_READ_ACCUMULATOR would
    cost ~285ns of ACT time per activation).
  - Class-sorted rows make same-class keys contiguous, so the numerator
    (own-class sum) only needs a 512-wide window around the diagonal,
    handled by mask-multiply + reduce on DVE with small host-built masks.
  - Proto terms, per-row weights, logs and the final partition reduction
    are a tiny epilogue; each core returns sum(loss_rows) over its rows.
  - Host sums the 8 partials and divides by 8192. No device collectives.
"""

import sys
import types

sys.path.insert(0, "/opt/trn_rl_repo")

import numpy as np

TAU = 0.1
EPS_FREQ = 1e-06
EPS_DENOM = 1e-12

N_CORES = 8
M = 8192          # total rows (2*4096)
D = 128           # feature dim
C = 64            # num classes
Q = M // N_CORES  # 1024 query rows per core
NQT = Q // 128    # 8 query tiles per core
CHUNK = 2048      # key chunk per ACT instruction
NCH = M // CHUNK  # 4 chunks per q-tile
HK = M // 2       # keysT is split in two DMA halves
W = 512           # band window width (own-class mates live here)
QOFF = 256        # own queries sit at rolled cols [QOFF, QOFF+Q)
WMARG = 192       # window starts at q-tile start - WMARG


def _install_ntff_hook():
    """Shim antenv.axon_hooks (absent in this image) so trace=True works."""
    if "antenv.axon_hooks" in sys.modules:
        return
    try:
        if "/root/.axon_site" not in sys.path:
            sys.path.insert(0, "/root/.axon_site")
        import trn_agent_boot.trn_boot as tb

        hook = tb._ntff_profile_via_ctypes("/opt/axon/libaxon_pjrt.so")
        mod = types.ModuleType("antenv.axon_hooks")
        mod._hook = hook
        mod.get_axon_ntff_profile_hook = lambda: mod._hook
        mod.set_axon_ntff_profile_hook = lambda h: setattr(mod, "_hook", h)
        sys.modules["antenv.axon_hooks"] = mod
        import antenv

        antenv.axon_hooks = mod
    except Exception:
        pass


def build_nc():
    """Build and compile the single-core Bass program (same NEFF on all 8)."""
    import concourse.bass as bass  # noqa: F401
    import concourse.mybir as mybir
    import concourse.bacc as bacc
    from concourse import tile

    f32 = mybir.dt.float32
    bf16 = mybir.dt.bfloat16
    mult = mybir.AluOpType.mult
    add = mybir.AluOpType.add
    sub = mybir.AluOpType.subtract
    Act = mybir.ActivationFunctionType

    nc = bacc.Bacc("TRN2", target_bir_lowering=False, debug=False,
                   num_devices=N_CORES)

    d_keysT = [nc.dram_tensor(f"keysT{h}", [128, HK], bf16,
                              kind="ExternalInput") for h in range(2)]
    d_keysA = nc.dram_tensor("keysA", [128, 2304], bf16,
                             kind="ExternalInput")
    d_protosT = nc.dram_tensor("protosT", [128, C], bf16,
                               kind="ExternalInput")
    d_mdiag = nc.dram_tensor("mdiag", [128, W], bf16, kind="ExternalInput")
    d_mclass = nc.dram_tensor("mclass", [128, NQT, W], bf16,
                              kind="ExternalInput")
    d_fwinv = nc.dram_tensor("fwinv", [128, NQT], f32, kind="ExternalInput")
    d_ohp = nc.dram_tensor("ohp", [128, NQT, C], f32, kind="ExternalInput")
    d_cfb = nc.dram_tensor("cfb", [128, NQT, C], f32, kind="ExternalInput")
    d_out = nc.dram_tensor("out", [1, 1], f32, kind="ExternalOutput")

    with tile.TileContext(nc) as tc:
        with (
            tc.tile_pool(name="const", bufs=1) as cst,
            tc.tile_pool(name="etring", bufs=4) as etring,
        ):
            keysT = [cst.tile([128, HK], bf16, tag=f"keysT{h}",
                              name=f"keysT{h}") for h in range(2)]
            keysA = cst.tile([128, 2304], bf16, tag="keysA")
            protosT = cst.tile([128, C], bf16, tag="protosT")
            mdiag = cst.tile([128, W], bf16, tag="mdiag")
            mclass = cst.tile([128, NQT, W], bf16, tag="mclass")
            fwinv = cst.tile([128, NQT], f32, tag="fwinv")
            ohp = cst.tile([128, NQT, C], f32, tag="ohp")
            cfb = cst.tile([128, NQT, C], f32, tag="cfb")

            # warm the ACT table while input DMAs stream
            warm = cst.tile([1, 1], f32, tag="warm")
            nc.vector.memset(warm[:], 0.0)
            wj = cst.tile([1, 1], f32, tag="wj")
            nc.scalar.activation(wj[:], warm[:], Act.Exp)

            nc.sync.dma_start(keysA[:], d_keysA[:])
            nc.sync.dma_start(keysT[0][:], d_keysT[0][:])
            nc.sync.dma_start(keysT[1][:], d_keysT[1][:])
            nc.sync.dma_start(protosT[:], d_protosT[:])
            nc.sync.dma_start(mdiag[:], d_mdiag[:])
            nc.sync.dma_start(mclass[:], d_mclass[:])
            nc.sync.dma_start(fwinv[:], d_fwinv[:])
            nc.sync.dma_start(ohp[:], d_ohp[:])
            nc.sync.dma_start(cfb[:], d_cfb[:])

            # accumulators / epilogue operands
            racc = [cst.tile([128, NQT], f32, tag=f"racc{i}",
                             name=f"racc{i}")
                    for i in range(NCH)]
            dsub = cst.tile([128, NQT], f32, tag="dsub")
            own = cst.tile([128, NQT], f32, tag="own")
            nprot = cst.tile([128, NQT, 1], f32, tag="nprot")
            dprot = cst.tile([128, NQT, 1], f32, tag="dprot")
            etp = cst.tile([128, NQT, C], f32, tag="etp")
            junkp = cst.tile([128, NQT, C], f32, tag="junkp")
            junkw = cst.tile([128, W], bf16, tag="junkw")
            onescol = cst.tile([128, 1], f32, tag="onescol")
            nc.vector.memset(onescol[:], 1.0)

            # ---- proto similarities for own queries: [128q, 8, 64] ----
            with tc.tile_pool(name="pp", bufs=1, space="PSUM") as ppool:
                pp = ppool.tile([128, NQT, C], f32, tag="pp")
                for t in range(NQT):
                    qc = QOFF + 128 * t
                    nc.tensor.matmul(pp[:, t], keysA[:, qc:qc + 128],
                                     protosT[:], start=True, stop=True)
                nc.scalar.activation(etp[:], pp[:], Act.Exp, scale=1.0 / TAU)
            # batched proto selects: one mult + one innermost-axis reduce each
            nc.vector.tensor_tensor(junkp[:], etp[:], ohp[:], op=mult)
            nc.vector.reduce_sum(nprot[:], junkp[:], axis=mybir.AxisListType.X)
            nc.vector.tensor_tensor(junkp[:], etp[:], cfb[:], op=mult)
            nc.vector.reduce_sum(dprot[:], junkp[:], axis=mybir.AxisListType.X)

            # ---- main loop: 8 q-tiles x 4 key chunks of 2048 ----
            with tc.tile_pool(name="ring", bufs=2, space="PSUM") as ring:
                for t in range(NQT):
                    qc = QOFF + 128 * t
                    for ch in range(NCH):
                        ps = ring.tile([128, CHUNK], f32, tag="ps")
                        for j in range(CHUNK // 512):
                            cb = CHUNK * ch + 512 * j
                            mv = keysA if cb + 512 <= 2304 \
                                else keysT[cb // HK]
                            mo = cb if cb + 512 <= 2304 else cb % HK
                            nc.tensor.matmul(ps[:, 512 * j:512 * (j + 1)],
                                             keysA[:, qc:qc + 128],
                                             mv[:, mo:mo + 512],
                                             start=True, stop=True)
                        et = etring.tile([128, CHUNK], bf16, tag="et")
                        # split row-sum work: half via ACT accum_out
                        # (hidden RAA costs ~285ns of ACT), half via DVE
                        # reduce (2.2us each) - keeps both engines under
                        # the exp roofline
                        on_act = t == NQT - 1 or (NCH * t + ch) % 2 == 1
                        if on_act:
                            nc.scalar.activation(
                                et[:], ps[:], Act.Exp, scale=1.0 / TAU,
                                accum_out=racc[ch][:, t:t + 1])
                        else:
                            nc.scalar.activation(et[:], ps[:], Act.Exp,
                                                 scale=1.0 / TAU)
                        if ch == 0:
                            # band window: diag value + own-class sum on DVE
                            win = slice(64 + 128 * t, 64 + 128 * t + W)
                            nc.vector.tensor_tensor(junkw[:], et[:, win],
                                                    mdiag[:], op=mult)
                            nc.vector.reduce_sum(dsub[:, t:t + 1], junkw[:],
                                                 axis=mybir.AxisListType.X)
                            nc.vector.tensor_tensor(junkw[:], et[:, win],
                                                    mclass[:, t], op=mult)
                            nc.vector.reduce_sum(own[:, t:t + 1], junkw[:],
                                                 axis=mybir.AxisListType.X)
                        if not on_act:
                            nc.vector.reduce_sum(racc[ch][:, t:t + 1], et[:],
                                                 axis=mybir.AxisListType.X)

            # ---- epilogue ----
            rs = cst.tile([128, NQT], f32, tag="rs")
            nc.vector.tensor_tensor(rs[:], racc[0][:], racc[1][:], op=add)
            for i in range(2, NCH):
                nc.vector.tensor_tensor(rs[:], rs[:], racc[i][:], op=add)
            nc.vector.tensor_tensor(rs[:], rs[:], dsub[:], op=sub)

            den = cst.tile([128, NQT], f32, tag="den")
            nc.vector.tensor_tensor(den[:], rs[:], fwinv[:], op=mult)
            nc.vector.tensor_tensor(den[:], den[:], dprot[:, :, 0], op=add)
            nc.vector.tensor_scalar_add(den[:], den[:], EPS_DENOM)
            num = cst.tile([128, NQT], f32, tag="num")
            nc.vector.tensor_tensor(num[:], own[:], nprot[:, :, 0], op=add)

            lbuf = cst.tile([128, NQT], f32, tag="lbuf")
            l1 = cst.tile([128, 1], f32, tag="l1")
            l2 = cst.tile([128, 1], f32, tag="l2")
            nc.scalar.activation(lbuf[:], den[:], Act.Ln, accum_out=l1[:])
            nc.scalar.activation(lbuf[:], num[:], Act.Ln, accum_out=l2[:])
            diff = cst.tile([128, 1], f32, tag="diff")
            nc.vector.tensor_tensor(diff[:], l1[:], l2[:], op=sub)

            with tc.tile_pool(name="pf", bufs=1, space="PSUM") as pfp:
                pf = pfp.tile([1, 1], f32, tag="pf")
                nc.tensor.matmul(pf[:], onescol[:], diff[:],
                                 start=True, stop=True)
                res = cst.tile([1, 1], f32, tag="res")
                nc.vector.tensor_copy(res[:], pf[:])
                nc.sync.dma_start(d_out[:], res[:])

    nc.compile()
    return nc


def make_in_maps(protos, proj2, target2, proj3, target3):
    import ml_dtypes

    bf16 = ml_dtypes.bfloat16
    f32 = np.float32

    feats = np.concatenate([np.asarray(proj2, dtype=f32),
                            np.asarray(proj3, dtype=f32)], axis=0)
    labels = np.concatenate([np.asarray(target2), np.asarray(target3)],
                            axis=0).astype(np.int64)

    order = np.argsort(labels, kind="stable")
    fs = feats[order]
    ls = labels[order]
    nrm = np.sqrt((fs * fs).sum(axis=1, keepdims=True))
    fn = fs / np.maximum(nrm, f32(1e-12))

    counts = np.bincount(ls, minlength=C).astype(f32)
    # class-mates of any row must fit the [start-WMARG, end+WMARG] window
    assert counts.max() <= WMARG + 1, "class count exceeds band window"
    cls_freq = (counts + f32(1.0)) + f32(EPS_FREQ)
    cfr = (f32(1.0) / cls_freq).astype(f32)

    pr = np.asarray(protos, dtype=f32)
    pnrm = np.sqrt((pr * pr).sum(axis=1, keepdims=True))
    pn = pr / np.maximum(pnrm, f32(1e-12))
    protosT = np.ascontiguousarray(pn.T).astype(bf16)

    mdiag = np.zeros((128, W), dtype=bf16)
    mdiag[np.arange(128), np.arange(128) + WMARG] = bf16(1.0)
    cfb = np.ascontiguousarray(
        np.broadcast_to(cfr, (128, NQT, C))).astype(f32)

    in_maps = []
    for c in range(N_CORES):
        roll = (Q * c - QOFF) % M
        idx = (np.arange(M) + roll) % M
        kf = fn[idx]
        kl = ls[idx]

        keysT = np.ascontiguousarray(kf.T).astype(bf16)

        mclass = np.zeros((128, NQT, W), dtype=bf16)
        fwinv = np.zeros((128, NQT), dtype=f32)
        ohp = np.zeros((128, NQT, C), dtype=f32)
        for t in range(NQT):
            rows = kl[QOFF + 128 * t:QOFF + 128 * t + 128]
            win = kl[64 + 128 * t:64 + 128 * t + W]
            mc = rows[:, None] == win[None, :]
            mc[np.arange(128), np.arange(128) + WMARG] = False
            mclass[:, t, :] = mc.astype(bf16)
            fwinv[:, t] = cfr[rows]
            ohp[np.arange(128), t, rows] = f32(1.0)

        in_maps.append({
            "keysA": np.ascontiguousarray(keysT[:, :2304]),
            "keysT0": np.ascontiguousarray(keysT[:, :HK]),
            "keysT1": np.ascontiguousarray(keysT[:, HK:]),
            "protosT": protosT,
            "mdiag": mdiag,
            "mclass": mclass,
            "fwinv": fwinv,
            "ohp": ohp,
            "cfb": cfb,
        })
    return in_maps


def run(in_maps, trace=False):
    _install_ntff_hook()
    from concourse import bass_utils

    nc = build_nc()
    res = bass_utils.run_bass_kernel_spmd(
        nc, in_maps, core_ids=list(range(N_CORES)), trace=trace)
    return res


def kernel(protos, proj2, target2, proj3, target3):
    in_maps = make_in_maps(protos, proj2, target2, proj3, target3)
    res = run(in_maps, trace=False)
    parts = [res.results[i]["out"][0, 0] for i in range(N_CORES)]
    total = np.sum(np.asarray(parts, dtype=np.float32))
    return np.asarray(total / np.float32(M), dtype=np.float32)


# revision 20
# speedup vs baseline: 1.0177x; 1.0177x over previous
"""Trainium2 Bass kernel for CropConLoss (supervised-contrastive style loss).

Contract: kernel(**inputs) takes the FULL unsharded inputs
(protos [64,128] f32, proj2/proj3 [4096,128] f32, target2/target3 [4096] i64)
and returns the FULL output (scalar f32 mean loss), running the compute on
8 NeuronCores via bass_utils.run_bass_kernel_spmd.

Strategy (data-parallel over the M=8192 rows of feats, ACT-roofline design):
  - Host sorts the 8192 rows by class label and l2-normalizes them (and the
    protos) in f32, so the device needs no sqrt/reciprocal and a single
    constant exp scale of 1/tau.
  - Each core owns 1024 query rows. Layout is [query-partition, key-free]:
    per q-tile (128 queries) the stationary operand is the query block of
    keysT and the 8192 keys stream through the PE in 512-col chunks.
  - exp runs on ACT over [128, 2048] PSUM chunks into f32 SBUF tiles; ACT
    is the roofline engine (~64us of exp). Row-sums are per-chunk DVE
    reduces (NOT accum_out - its hidden ACTIVATION_READ_ACCUMULATOR would
    cost ~285ns of ACT time per activation).
  - Class-sorted rows make same-class keys contiguous, so the numerator
    (own-class sum) only needs a 512-wide window around the diagonal,
    handled by mask-multiply + reduce on DVE with small host-built masks.
  - Proto terms, per-row weights, logs and the final partition reduction
    are a tiny epilogue; each core returns sum(loss_rows) over its rows.
  - Host sums the 8 partials and divides by 8192. No device collectives.
"""

import sys
import types

sys.path.insert(0, "/opt/trn_rl_repo")

import numpy as np

TAU = 0.1
EPS_FREQ = 1e-06
EPS_DENOM = 1e-12

N_CORES = 8
M = 8192          # total rows (2*4096)
D = 128           # feature dim
C = 64            # num classes
Q = M // N_CORES  # 1024 query rows per core
NQT = Q // 128    # 8 query tiles per core
CHUNK = 2048      # key chunk per ACT instruction
NCH = M // CHUNK  # 4 chunks per q-tile
HK = M // 2       # keysT is split in two DMA halves
W = 512           # band window width (own-class mates live here)
QOFF = 256        # own queries sit at rolled cols [QOFF, QOFF+Q)
WMARG = 192       # window starts at q-tile start - WMARG


def _install_ntff_hook():
    """Shim antenv.axon_hooks (absent in this image) so trace=True works."""
    if "antenv.axon_hooks" in sys.modules:
        return
    try:
        if "/root/.axon_site" not in sys.path:
            sys.path.insert(0, "/root/.axon_site")
        import trn_agent_boot.trn_boot as tb

        hook = tb._ntff_profile_via_ctypes("/opt/axon/libaxon_pjrt.so")
        mod = types.ModuleType("antenv.axon_hooks")
        mod._hook = hook
        mod.get_axon_ntff_profile_hook = lambda: mod._hook
        mod.set_axon_ntff_profile_hook = lambda h: setattr(mod, "_hook", h)
        sys.modules["antenv.axon_hooks"] = mod
        import antenv

        antenv.axon_hooks = mod
    except Exception:
        pass


def build_nc():
    """Build and compile the single-core Bass program (same NEFF on all 8)."""
    import concourse.bass as bass  # noqa: F401
    import concourse.mybir as mybir
    import concourse.bacc as bacc
    from concourse import tile

    f32 = mybir.dt.float32
    bf16 = mybir.dt.bfloat16
    mult = mybir.AluOpType.mult
    add = mybir.AluOpType.add
    sub = mybir.AluOpType.subtract
    Act = mybir.ActivationFunctionType

    nc = bacc.Bacc("TRN2", target_bir_lowering=False, debug=False,
                   num_devices=N_CORES)

    d_keysT = [nc.dram_tensor(f"keysT{h}", [128, HK], bf16,
                              kind="ExternalInput") for h in range(2)]
    d_keysA = nc.dram_tensor("keysA", [128, 2304], bf16,
                             kind="ExternalInput")
    d_protosT = nc.dram_tensor("protosT", [128, C], bf16,
                               kind="ExternalInput")
    d_mdiag = nc.dram_tensor("mdiag", [128, W], bf16, kind="ExternalInput")
    d_mclass = nc.dram_tensor("mclass", [128, NQT, W], bf16,
                              kind="ExternalInput")
    d_fwinv = nc.dram_tensor("fwinv", [128, NQT], f32, kind="ExternalInput")
    d_ohp = nc.dram_tensor("ohp", [128, NQT, C], f32, kind="ExternalInput")
    d_cfb = nc.dram_tensor("cfb", [128, NQT, C], f32, kind="ExternalInput")
    d_out = nc.dram_tensor("out", [1, 1], f32, kind="ExternalOutput")

    with tile.TileContext(nc) as tc:
        with (
            tc.tile_pool(name="const", bufs=1) as cst,
            tc.tile_pool(name="etring", bufs=4) as etring,
        ):
            keysT = [cst.tile([128, HK], bf16, tag=f"keysT{h}",
                              name=f"keysT{h}") for h in range(2)]
            keysA = cst.tile([128, 2304], bf16, tag="keysA")
            protosT = cst.tile([128, C], bf16, tag="protosT")
            mdiag = cst.tile([128, W], bf16, tag="mdiag")
            mclass = cst.tile([128, NQT, W], bf16, tag="mclass")
            fwinv = cst.tile([128, NQT], f32, tag="fwinv")
            ohp = cst.tile([128, NQT, C], f32, tag="ohp")
            cfb = cst.tile([128, NQT, C], f32, tag="cfb")

            # warm the ACT table while input DMAs stream
            warm = cst.tile([1, 1], f32, tag="warm")
            nc.vector.memset(warm[:], 0.0)
            wj = cst.tile([1, 1], f32, tag="wj")
            nc.scalar.activation(wj[:], warm[:], Act.Exp)

            nc.sync.dma_start(keysA[:], d_keysA[:])
            nc.sync.dma_start(keysT[0][:], d_keysT[0][:])
            nc.sync.dma_start(keysT[1][:], d_keysT[1][:])
            nc.sync.dma_start(protosT[:], d_protosT[:])
            nc.sync.dma_start(mdiag[:], d_mdiag[:])
            nc.sync.dma_start(mclass[:], d_mclass[:])
            nc.sync.dma_start(fwinv[:], d_fwinv[:])
            nc.sync.dma_start(ohp[:], d_ohp[:])
            nc.sync.dma_start(cfb[:], d_cfb[:])

            # accumulators / epilogue operands
            racc = [cst.tile([128, NQT], f32, tag=f"racc{i}",
                             name=f"racc{i}")
                    for i in range(NCH)]
            dsub = cst.tile([128, NQT], f32, tag="dsub")
            own = cst.tile([128, NQT], f32, tag="own")
            nprot = cst.tile([128, NQT, 1], f32, tag="nprot")
            dprot = cst.tile([128, NQT, 1], f32, tag="dprot")
            etp = cst.tile([128, NQT, C], f32, tag="etp")
            junkp = cst.tile([128, NQT, C], f32, tag="junkp")
            junkw = cst.tile([128, W], bf16, tag="junkw")
            onescol = cst.tile([128, 1], f32, tag="onescol")
            nc.vector.memset(onescol[:], 1.0)

            # ---- main loop: 8 q-tiles x 4 key chunks of 2048 ----
            with tc.tile_pool(name="ring", bufs=2, space="PSUM") as ring:
                for t in range(NQT):
                    qc = QOFF + 128 * t
                    for ch in range(NCH):
                        ps = ring.tile([128, CHUNK], f32, tag="ps")
                        for j in range(CHUNK // 512):
                            cb = CHUNK * ch + 512 * j
                            mv = keysA if cb + 512 <= 2304 \
                                else keysT[cb // HK]
                            mo = cb if cb + 512 <= 2304 else cb % HK
                            nc.tensor.matmul(ps[:, 512 * j:512 * (j + 1)],
                                             keysA[:, qc:qc + 128],
                                             mv[:, mo:mo + 512],
                                             start=True, stop=True)
                        et = etring.tile([128, CHUNK], bf16, tag="et")
                        # split row-sum work: half via ACT accum_out
                        # (hidden RAA costs ~285ns of ACT), half via DVE
                        # reduce (2.2us each) - keeps both engines under
                        # the exp roofline
                        on_act = t == NQT - 1 or (NCH * t + ch) % 2 == 1
                        if on_act:
                            nc.scalar.activation(
                                et[:], ps[:], Act.Exp, scale=1.0 / TAU,
                                accum_out=racc[ch][:, t:t + 1])
                        else:
                            nc.scalar.activation(et[:], ps[:], Act.Exp,
                                                 scale=1.0 / TAU)
                        if ch == 0:
                            # band window: diag value + own-class sum on DVE
                            win = slice(64 + 128 * t, 64 + 128 * t + W)
                            nc.vector.tensor_tensor(junkw[:], et[:, win],
                                                    mdiag[:], op=mult)
                            nc.vector.reduce_sum(dsub[:, t:t + 1], junkw[:],
                                                 axis=mybir.AxisListType.X)
                            nc.vector.tensor_tensor(junkw[:], et[:, win],
                                                    mclass[:, t], op=mult)
                            nc.vector.reduce_sum(own[:, t:t + 1], junkw[:],
                                                 axis=mybir.AxisListType.X)
                        if not on_act:
                            nc.vector.reduce_sum(racc[ch][:, t:t + 1], et[:],
                                                 axis=mybir.AxisListType.X)

            # ---- proto similarities (needed only by the epilogue) ----
            with tc.tile_pool(name="pp", bufs=1, space="PSUM") as ppool:
                pp = ppool.tile([128, NQT, C], f32, tag="pp")
                for t in range(NQT):
                    qc = QOFF + 128 * t
                    nc.tensor.matmul(pp[:, t], keysA[:, qc:qc + 128],
                                     protosT[:], start=True, stop=True)
                nc.scalar.activation(etp[:], pp[:], Act.Exp, scale=1.0 / TAU)
            # batched proto selects: one mult + one innermost-axis reduce each
            nc.vector.tensor_tensor(junkp[:], etp[:], ohp[:], op=mult)
            nc.vector.reduce_sum(nprot[:], junkp[:], axis=mybir.AxisListType.X)
            nc.vector.tensor_tensor(junkp[:], etp[:], cfb[:], op=mult)
            nc.vector.reduce_sum(dprot[:], junkp[:], axis=mybir.AxisListType.X)

            # ---- epilogue ----
            rs = cst.tile([128, NQT], f32, tag="rs")
            nc.vector.tensor_tensor(rs[:], racc[0][:], racc[1][:], op=add)
            for i in range(2, NCH):
                nc.vector.tensor_tensor(rs[:], rs[:], racc[i][:], op=add)
            nc.vector.tensor_tensor(rs[:], rs[:], dsub[:], op=sub)

            den = cst.tile([128, NQT], f32, tag="den")
            nc.vector.tensor_tensor(den[:], rs[:], fwinv[:], op=mult)
            nc.vector.tensor_tensor(den[:], den[:], dprot[:, :, 0], op=add)
            nc.vector.tensor_scalar_add(den[:], den[:], EPS_DENOM)
            num = cst.tile([128, NQT], f32, tag="num")
            nc.vector.tensor_tensor(num[:], own[:], nprot[:, :, 0], op=add)

            lbuf = cst.tile([128, NQT], f32, tag="lbuf")
            l1 = cst.tile([128, 1], f32, tag="l1")
            l2 = cst.tile([128, 1], f32, tag="l2")
            nc.scalar.activation(lbuf[:], den[:], Act.Ln, accum_out=l1[:])
            nc.scalar.activation(lbuf[:], num[:], Act.Ln, accum_out=l2[:])
            diff = cst.tile([128, 1], f32, tag="diff")
            nc.vector.tensor_tensor(diff[:], l1[:], l2[:], op=sub)

            with tc.tile_pool(name="pf", bufs=1, space="PSUM") as pfp:
                pf = pfp.tile([1, 1], f32, tag="pf")
                nc.tensor.matmul(pf[:], onescol[:], diff[:],
                                 start=True, stop=True)
                res = cst.tile([1, 1], f32, tag="res")
                nc.vector.tensor_copy(res[:], pf[:])
                nc.sync.dma_start(d_out[:], res[:])

    nc.compile()
    return nc


def make_in_maps(protos, proj2, target2, proj3, target3):
    import ml_dtypes

    bf16 = ml_dtypes.bfloat16
    f32 = np.float32

    feats = np.concatenate([np.asarray(proj2, dtype=f32),
                            np.asarray(proj3, dtype=f32)], axis=0)
    labels = np.concatenate([np.asarray(target2), np.asarray(target3)],
                            axis=0).astype(np.int64)

    order = np.argsort(labels, kind="stable")
    fs = feats[order]
    ls = labels[order]
    nrm = np.sqrt((fs * fs).sum(axis=1, keepdims=True))
    fn = fs / np.maximum(nrm, f32(1e-12))

    counts = np.bincount(ls, minlength=C).astype(f32)
    # class-mates of any row must fit the [start-WMARG, end+WMARG] window
    assert counts.max() <= WMARG + 1, "class count exceeds band window"
    cls_freq = (counts + f32(1.0)) + f32(EPS_FREQ)
    cfr = (f32(1.0) / cls_freq).astype(f32)

    pr = np.asarray(protos, dtype=f32)
    pnrm = np.sqrt((pr * pr).sum(axis=1, keepdims=True))
    pn = pr / np.maximum(pnrm, f32(1e-12))
    protosT = np.ascontiguousarray(pn.T).astype(bf16)

    mdiag = np.zeros((128, W), dtype=bf16)
    mdiag[np.arange(128), np.arange(128) + WMARG] = bf16(1.0)
    cfb = np.ascontiguousarray(
        np.broadcast_to(cfr, (128, NQT, C))).astype(f32)

    in_maps = []
    for c in range(N_CORES):
        roll = (Q * c - QOFF) % M
        idx = (np.arange(M) + roll) % M
        kf = fn[idx]
        kl = ls[idx]

        keysT = np.ascontiguousarray(kf.T).astype(bf16)

        mclass = np.zeros((128, NQT, W), dtype=bf16)
        fwinv = np.zeros((128, NQT), dtype=f32)
        ohp = np.zeros((128, NQT, C), dtype=f32)
        for t in range(NQT):
            rows = kl[QOFF + 128 * t:QOFF + 128 * t + 128]
            win = kl[64 + 128 * t:64 + 128 * t + W]
            mc = rows[:, None] == win[None, :]
            mc[np.arange(128), np.arange(128) + WMARG] = False
            mclass[:, t, :] = mc.astype(bf16)
            fwinv[:, t] = cfr[rows]
            ohp[np.arange(128), t, rows] = f32(1.0)

        in_maps.append({
            "keysA": np.ascontiguousarray(keysT[:, :2304]),
            "keysT0": np.ascontiguousarray(keysT[:, :HK]),
            "keysT1": np.ascontiguousarray(keysT[:, HK:]),
            "protosT": protosT,
            "mdiag": mdiag,
            "mclass": mclass,
            "fwinv": fwinv,
            "ohp": ohp,
            "cfb": cfb,
        })
    return in_maps


def run(in_maps, trace=False):
    _install_ntff_hook()
    from concourse import bass_utils

    nc = build_nc()
    res = bass_utils.run_bass_kernel_spmd(
        nc, in_maps, core_ids=list(range(N_CORES)), trace=trace)
    return res


def kernel(protos, proj2, target2, proj3, target3):
    in_maps = make_in_maps(protos, proj2, target2, proj3, target3)
    res = run(in_maps, trace=False)
    parts = [res.results[i]["out"][0, 0] for i in range(N_CORES)]
    total = np.sum(np.asarray(parts, dtype=np.float32))
    return np.asarray(total / np.float32(M), dtype=np.float32)


# revision 21
# speedup vs baseline: 1.0378x; 1.0198x over previous
"""Trainium2 Bass kernel for CropConLoss (supervised-contrastive style loss).

Contract: kernel(**inputs) takes the FULL unsharded inputs
(protos [64,128] f32, proj2/proj3 [4096,128] f32, target2/target3 [4096] i64)
and returns the FULL output (scalar f32 mean loss), running the compute on
8 NeuronCores via bass_utils.run_bass_kernel_spmd.

Strategy (data-parallel over the M=8192 rows of feats, ACT-roofline design):
  - Host sorts the 8192 rows by class label and l2-normalizes them (and the
    protos) in f32, so the device needs no sqrt/reciprocal and a single
    constant exp scale of 1/tau.
  - Each core owns 1024 query rows. Layout is [query-partition, key-free]:
    per q-tile (128 queries) the stationary operand is the query block of
    keysT (bf16) and the 8192 keys stream through the PE in 512-col chunks
    into a 2-deep PSUM ring of [128, 2048] f32 tiles.
  - exp runs on ACT over the [128, 2048] PSUM chunks into bf16 SBUF tiles;
    ACT is the roofline engine (~63us of exp at 1 elem/cycle/lane).
  - Per-row key-sums are split between the two engines that can do them:
    ~half via ACT accum_out (whose hidden ACTIVATION_READ_ACCUMULATOR
    costs ~285ns of ACT each) and half via DVE free-axis reduces (~2.2us
    per chunk, dtype-independent) so neither engine exceeds the exp
    roofline. The last q-tile uses accum_out only, keeping DVE off the
    kernel tail.
  - Class-sorted rows make same-class keys contiguous, so the numerator
    (own-class sum) only needs a 512-wide window around the diagonal,
    handled by mask-multiply + reduce on DVE with small host-built masks
    (diag-only mask recovers the exact diagonal term to subtract from the
    denominator row-sum; class-mask-minus-diag gives the numerator).
  - A small leading slice of keysT (keysA) gets its own DMA so the first
    matmuls/exps start before the full 2MB key matrix lands; the proto
    similarity terms (needed only by the epilogue) run after the main
    loop where PE is idle.
  - Epilogue: per-row weights, logs (exp and ln share one ACT table set,
    warmed at kernel start), and a ones-matmul partition reduction; each
    core returns sum(loss_rows) over its 1024 rows.
  - Host sums the 8 partials and divides by 8192. No device collectives.

Known pitfalls encoded here: tensor_tensor_reduce passes CoreSim but
kills the device on this toolchain (split into tensor_tensor+reduce_sum);
GpSimd tensor_reduce cannot reduce the free axis; DVE reduce throughput
does not double for bf16, but bf16 inputs still reduce with f32
accumulation (verified: rel err ~7e-6).
"""

import sys
import types

sys.path.insert(0, "/opt/trn_rl_repo")

import numpy as np

TAU = 0.1
EPS_FREQ = 1e-06
EPS_DENOM = 1e-12

N_CORES = 8
M = 8192          # total rows (2*4096)
D = 128           # feature dim
C = 64            # num classes
Q = M // N_CORES  # 1024 query rows per core
NQT = Q // 128    # 8 query tiles per core
CHUNK = 2048      # key chunk per ACT instruction
NCH = M // CHUNK  # 4 chunks per q-tile
HK = M // 2       # keysT is split in two DMA halves
W = 512           # band window width (own-class mates live here)
QOFF = 256        # own queries sit at rolled cols [QOFF, QOFF+Q)
WMARG = 192       # window starts at q-tile start - WMARG


def _install_ntff_hook():
    """Shim antenv.axon_hooks (absent in this image) so trace=True works."""
    if "antenv.axon_hooks" in sys.modules:
        return
    try:
        if "/root/.axon_site" not in sys.path:
            sys.path.insert(0, "/root/.axon_site")
        import trn_agent_boot.trn_boot as tb

        hook = tb._ntff_profile_via_ctypes("/opt/axon/libaxon_pjrt.so")
        mod = types.ModuleType("antenv.axon_hooks")
        mod._hook = hook
        mod.get_axon_ntff_profile_hook = lambda: mod._hook
        mod.set_axon_ntff_profile_hook = lambda h: setattr(mod, "_hook", h)
        sys.modules["antenv.axon_hooks"] = mod
        import antenv

        antenv.axon_hooks = mod
    except Exception:
        pass


def build_nc():
    """Build and compile the single-core Bass program (same NEFF on all 8)."""
    import concourse.bass as bass  # noqa: F401
    import concourse.mybir as mybir
    import concourse.bacc as bacc
    from concourse import tile

    f32 = mybir.dt.float32
    bf16 = mybir.dt.bfloat16
    mult = mybir.AluOpType.mult
    add = mybir.AluOpType.add
    sub = mybir.AluOpType.subtract
    Act = mybir.ActivationFunctionType

    nc = bacc.Bacc("TRN2", target_bir_lowering=False, debug=False,
                   num_devices=N_CORES)

    d_keysT = [nc.dram_tensor(f"keysT{h}", [128, HK], bf16,
                              kind="ExternalInput") for h in range(2)]
    d_keysA = nc.dram_tensor("keysA", [128, 2304], bf16,
                             kind="ExternalInput")
    d_protosT = nc.dram_tensor("protosT", [128, C], bf16,
                               kind="ExternalInput")
    d_mdiag = nc.dram_tensor("mdiag", [128, W], bf16, kind="ExternalInput")
    d_mclass = nc.dram_tensor("mclass", [128, NQT, W], bf16,
                              kind="ExternalInput")
    d_fwinv = nc.dram_tensor("fwinv", [128, NQT], f32, kind="ExternalInput")
    d_ohp = nc.dram_tensor("ohp", [128, NQT, C], f32, kind="ExternalInput")
    d_cfb = nc.dram_tensor("cfb", [128, NQT, C], f32, kind="ExternalInput")
    d_out = nc.dram_tensor("out", [1, 1], f32, kind="ExternalOutput")

    with tile.TileContext(nc) as tc:
        with (
            tc.tile_pool(name="const", bufs=1) as cst,
            tc.tile_pool(name="etring", bufs=4) as etring,
        ):
            keysT = [cst.tile([128, HK], bf16, tag=f"keysT{h}",
                              name=f"keysT{h}") for h in range(2)]
            keysA = cst.tile([128, 2304], bf16, tag="keysA")
            protosT = cst.tile([128, C], bf16, tag="protosT")
            mdiag = cst.tile([128, W], bf16, tag="mdiag")
            mclass = cst.tile([128, NQT, W], bf16, tag="mclass")
            fwinv = cst.tile([128, NQT], f32, tag="fwinv")
            ohp = cst.tile([128, NQT, C], f32, tag="ohp")
            cfb = cst.tile([128, NQT, C], f32, tag="cfb")

            # warm the ACT table while input DMAs stream
            warm = cst.tile([1, 1], f32, tag="warm")
            nc.vector.memset(warm[:], 0.0)
            wj = cst.tile([1, 1], f32, tag="wj")
            nc.scalar.activation(wj[:], warm[:], Act.Exp)

            nc.sync.dma_start(keysA[:], d_keysA[:])
            nc.sync.dma_start(keysT[0][:], d_keysT[0][:])
            nc.sync.dma_start(keysT[1][:], d_keysT[1][:])
            nc.sync.dma_start(protosT[:], d_protosT[:])
            nc.sync.dma_start(mdiag[:], d_mdiag[:])
            nc.sync.dma_start(mclass[:], d_mclass[:])
            nc.sync.dma_start(fwinv[:], d_fwinv[:])
            nc.sync.dma_start(ohp[:], d_ohp[:])
            nc.sync.dma_start(cfb[:], d_cfb[:])

            # accumulators / epilogue operands
            racc = [cst.tile([128, NQT], f32, tag=f"racc{i}",
                             name=f"racc{i}")
                    for i in range(NCH)]
            dsub = cst.tile([128, NQT], f32, tag="dsub")
            own = cst.tile([128, NQT], f32, tag="own")
            nprot = cst.tile([128, NQT, 1], f32, tag="nprot")
            dprot = cst.tile([128, NQT, 1], f32, tag="dprot")
            etp = cst.tile([128, NQT, C], f32, tag="etp")
            junkp = cst.tile([128, NQT, C], f32, tag="junkp")
            junkw = cst.tile([128, W], bf16, tag="junkw")
            onescol = cst.tile([128, 1], f32, tag="onescol")
            nc.vector.memset(onescol[:], 1.0)

            # ---- main loop: 8 q-tiles x 4 key chunks of 2048 ----
            with tc.tile_pool(name="ring", bufs=2, space="PSUM") as ring:
                for t in range(NQT):
                    qc = QOFF + 128 * t
                    for ch in range(NCH):
                        ps = ring.tile([128, CHUNK], f32, tag="ps")
                        for j in range(CHUNK // 512):
                            cb = CHUNK * ch + 512 * j
                            mv = keysA if cb + 512 <= 2304 \
                                else keysT[cb // HK]
                            mo = cb if cb + 512 <= 2304 else cb % HK
                            nc.tensor.matmul(ps[:, 512 * j:512 * (j + 1)],
                                             keysA[:, qc:qc + 128],
                                             mv[:, mo:mo + 512],
                                             start=True, stop=True)
                        et = etring.tile([128, CHUNK], bf16, tag="et")
                        # split row-sum work: half via ACT accum_out
                        # (hidden RAA costs ~285ns of ACT), half via DVE
                        # reduce (2.2us each) - keeps both engines under
                        # the exp roofline
                        on_act = t == NQT - 1 or (NCH * t + ch) % 2 == 1
                        if on_act:
                            nc.scalar.activation(
                                et[:], ps[:], Act.Exp, scale=1.0 / TAU,
                                accum_out=racc[ch][:, t:t + 1])
                        else:
                            nc.scalar.activation(et[:], ps[:], Act.Exp,
                                                 scale=1.0 / TAU)
                        if ch == 0:
                            # band window: diag value + own-class sum on DVE
                            win = slice(64 + 128 * t, 64 + 128 * t + W)
                            nc.vector.tensor_tensor(junkw[:], et[:, win],
                                                    mdiag[:], op=mult)
                            nc.vector.reduce_sum(dsub[:, t:t + 1], junkw[:],
                                                 axis=mybir.AxisListType.X)
                            nc.vector.tensor_tensor(junkw[:], et[:, win],
                                                    mclass[:, t], op=mult)
                            nc.vector.reduce_sum(own[:, t:t + 1], junkw[:],
                                                 axis=mybir.AxisListType.X)
                        if not on_act:
                            nc.vector.reduce_sum(racc[ch][:, t:t + 1], et[:],
                                                 axis=mybir.AxisListType.X)

            # ---- proto similarities (needed only by the epilogue) ----
            with tc.tile_pool(name="pp", bufs=1, space="PSUM") as ppool:
                pp = ppool.tile([128, NQT, C], f32, tag="pp")
                for t in range(NQT):
                    qc = QOFF + 128 * t
                    nc.tensor.matmul(pp[:, t], keysA[:, qc:qc + 128],
                                     protosT[:], start=True, stop=True)
                nc.scalar.activation(etp[:], pp[:], Act.Exp, scale=1.0 / TAU)
            # batched proto selects: one mult + one innermost-axis reduce each
            nc.vector.tensor_tensor(junkp[:], etp[:], ohp[:], op=mult)
            nc.vector.reduce_sum(nprot[:], junkp[:], axis=mybir.AxisListType.X)
            nc.vector.tensor_tensor(junkp[:], etp[:], cfb[:], op=mult)
            nc.vector.reduce_sum(dprot[:], junkp[:], axis=mybir.AxisListType.X)

            # ---- epilogue ----
            rs = cst.tile([128, NQT], f32, tag="rs")
            nc.vector.tensor_tensor(rs[:], racc[0][:], racc[1][:], op=add)
            for i in range(2, NCH):
                nc.vector.tensor_tensor(rs[:], rs[:], racc[i][:], op=add)
            nc.vector.tensor_tensor(rs[:], rs[:], dsub[:], op=sub)

            den = cst.tile([128, NQT], f32, tag="den")
            nc.vector.tensor_tensor(den[:], rs[:], fwinv[:], op=mult)
            nc.vector.tensor_tensor(den[:], den[:], dprot[:, :, 0], op=add)
            nc.vector.tensor_scalar_add(den[:], den[:], EPS_DENOM)
            num = cst.tile([128, NQT], f32, tag="num")
            nc.vector.tensor_tensor(num[:], own[:], nprot[:, :, 0], op=add)

            lbuf = cst.tile([128, NQT], f32, tag="lbuf")
            l1 = cst.tile([128, 1], f32, tag="l1")
            l2 = cst.tile([128, 1], f32, tag="l2")
            nc.scalar.activation(lbuf[:], den[:], Act.Ln, accum_out=l1[:])
            nc.scalar.activation(lbuf[:], num[:], Act.Ln, accum_out=l2[:])
            diff = cst.tile([128, 1], f32, tag="diff")
            nc.vector.tensor_tensor(diff[:], l1[:], l2[:], op=sub)

            with tc.tile_pool(name="pf", bufs=1, space="PSUM") as pfp:
                pf = pfp.tile([1, 1], f32, tag="pf")
                nc.tensor.matmul(pf[:], onescol[:], diff[:],
                                 start=True, stop=True)
                res = cst.tile([1, 1], f32, tag="res")
                nc.vector.tensor_copy(res[:], pf[:])
                nc.sync.dma_start(d_out[:], res[:])

    nc.compile()
    return nc


def make_in_maps(protos, proj2, target2, proj3, target3):
    import ml_dtypes

    bf16 = ml_dtypes.bfloat16
    f32 = np.float32

    feats = np.concatenate([np.asarray(proj2, dtype=f32),
                            np.asarray(proj3, dtype=f32)], axis=0)
    labels = np.concatenate([np.asarray(target2), np.asarray(target3)],
                            axis=0).astype(np.int64)

    order = np.argsort(labels, kind="stable")
    fs = feats[order]
    ls = labels[order]
    nrm = np.sqrt((fs * fs).sum(axis=1, keepdims=True))
    fn = fs / np.maximum(nrm, f32(1e-12))

    counts = np.bincount(ls, minlength=C).astype(f32)
    # class-mates of any row must fit the [start-WMARG, end+WMARG] window
    assert counts.max() <= WMARG + 1, "class count exceeds band window"
    cls_freq = (counts + f32(1.0)) + f32(EPS_FREQ)
    cfr = (f32(1.0) / cls_freq).astype(f32)

    pr = np.asarray(protos, dtype=f32)
    pnrm = np.sqrt((pr * pr).sum(axis=1, keepdims=True))
    pn = pr / np.maximum(pnrm, f32(1e-12))
    protosT = np.ascontiguousarray(pn.T).astype(bf16)

    mdiag = np.zeros((128, W), dtype=bf16)
    mdiag[np.arange(128), np.arange(128) + WMARG] = bf16(1.0)
    cfb = np.ascontiguousarray(
        np.broadcast_to(cfr, (128, NQT, C))).astype(f32)

    in_maps = []
    for c in range(N_CORES):
        roll = (Q * c - QOFF) % M
        idx = (np.arange(M) + roll) % M
        kf = fn[idx]
        kl = ls[idx]

        keysT = np.ascontiguousarray(kf.T).astype(bf16)

        mclass = np.zeros((128, NQT, W), dtype=bf16)
        fwinv = np.zeros((128, NQT), dtype=f32)
        ohp = np.zeros((128, NQT, C), dtype=f32)
        for t in range(NQT):
            rows = kl[QOFF + 128 * t:QOFF + 128 * t + 128]
            win = kl[64 + 128 * t:64 + 128 * t + W]
            mc = rows[:, None] == win[None, :]
            mc[np.arange(128), np.arange(128) + WMARG] = False
            mclass[:, t, :] = mc.astype(bf16)
            fwinv[:, t] = cfr[rows]
            ohp[np.arange(128), t, rows] = f32(1.0)

        in_maps.append({
            "keysA": np.ascontiguousarray(keysT[:, :2304]),
            "keysT0": np.ascontiguousarray(keysT[:, :HK]),
            "keysT1": np.ascontiguousarray(keysT[:, HK:]),
            "protosT": protosT,
            "mdiag": mdiag,
            "mclass": mclass,
            "fwinv": fwinv,
            "ohp": ohp,
            "cfb": cfb,
        })
    return in_maps


def run(in_maps, trace=False):
    _install_ntff_hook()
    from concourse import bass_utils

    nc = build_nc()
    res = bass_utils.run_bass_kernel_spmd(
        nc, in_maps, core_ids=list(range(N_CORES)), trace=trace)
    return res


def kernel(protos, proj2, target2, proj3, target3):
    in_maps = make_in_maps(protos, proj2, target2, proj3, target3)
    res = run(in_maps, trace=False)
    parts = [res.results[i]["out"][0, 0] for i in range(N_CORES)]
    total = np.sum(np.asarray(parts, dtype=np.float32))
    return np.asarray(total / np.float32(M), dtype=np.float32)
